# revision 1
# baseline (speedup 1.0000x reference)
"""Trainium2 Bass kernel for the GCNEncoder problem (v2).

Strategy (all-bf16, tile_position packing):
  - Pure data parallelism: batch 65536 = 8192 per core x 8 cores.
  - Host folding: C1 = kron(A,W1) (1088,51), C2 = kron(A,W2) block-sparse,
    D = Wp1 @ kron(A,W3) (64,1088); bp2 folded into stage D as a 65th
    contraction row (ones row in z4), output DMA'd straight from PSUM.
  - Stage A (h1): 9 blocks K=51 -> 2x row-tiled (tile_position (0,0)/(64,0)),
    x + g1 replicated at partitions 0-50 and 64-114.  5 slots.
  - Stage B (h2): C2 decomposed into 26 single-joint K=64 chunks using
    separate output/input joint pairings found by search; chunks balanced
    13 lo / 13 hi and row-tiled 2x -> ~13 slots (vs 18 full-block matmuls).
  - Stage C (z4): 9 accumulating K=128 matmuls, M=64.
  - Stage D: K=65 (bias row), 4 matmuls N=256, direct PSUM->HBM DMA.
  - Evacuations (bias+ReLU, fp32 PSUM -> bf16 SBUF) round-robin over
    ScalarE / VectorE / GpSimd.
"""

import os
import sys

for _p in ("/opt/trn_rl_repo", "/root/.axon_site/_ro/trn_rl_repo"):
    if os.path.isdir(_p) and _p not in sys.path:
        sys.path.insert(0, _p)

import numpy as np
import ml_dtypes

from concourse import bacc, mybir, tile
from concourse.bass_utils import run_bass_kernel_spmd

NJ = 17
DIN = 3
H = 64
DOUT = 256
NCORES = 8
B_TOTAL = 65536
BC = B_TOTAL // NCORES          # 8192
TILE_N = 512
NTILES = BC // TILE_N           # 16

F32 = mybir.dt.float32
BF16 = mybir.dt.bfloat16

# Output-side and input-side joint pairings for stage B (search-optimized:
# 26 K=64 chunks, 13 lo / 13 hi).
POUT = [(13, 16), (0, 2), (8, 10), (4, 6), (7, 11), (5, 15), (9, 14), (1, 3), (12,)]
PIN = [(2, 11), (1, 10), (12, 15), (7, 14), (5, 6), (9, 13), (3, 4), (0, 16), (8,)]

CONNS = [(0, 7), (7, 8), (8, 9), (9, 10), (0, 1), (1, 2), (2, 3), (0, 4),
         (4, 5), (5, 6), (8, 11), (11, 12), (12, 13), (8, 14), (14, 15), (15, 16)]

LAST_RESULTS = None


def _pin_index(j):
    for wi, p in enumerate(PIN):
        if j in p:
            return wi
    raise AssertionError(j)


def _plan():
    """Stage-B chunk list, lo/hi assignment, and a packed schedule.

    Returns:
      chunks: list of (q, j, half): output group q accumulates the K=64
              contribution of input joint j sitting at row-half `half`.
      halfof: dict j -> half (0 = partitions 0-63, 1 = 64-127).
      slots:  list of tuples of chunk indices (1 or 2, lo+hi, distinct q),
              ordered so that few output groups are open at a time.
    """
    nbr = [set() for _ in range(NJ)]
    for a, b in CONNS:
        nbr[a].add(b)
        nbr[b].add(a)

    raw = []
    for qi, q in enumerate(POUT):
        nq = set()
        for j in q:
            nq |= nbr[j]
        for j in sorted(nq):
            raw.append((qi, j))

    # Single-position groups: every chunk of an output group q runs at one
    # row position (HW: a group that switches row position mid-accumulation
    # crashes the device).  Classes chosen so the per-class input-joint sets
    # have size 9 each (one duplicated joint), preserving 9 h1 windows.
    LCLS = {1, 2, 5}          # groups whose chunks run at rows 0-63
    half_of_q = {q: (0 if q in LCLS else 1) for q in range(len(POUT))}

    lo_joints = sorted({j for q, j in raw if half_of_q[q] == 0})
    hi_joints = sorted({j for q, j in raw if half_of_q[q] == 1})
    assert len(lo_joints) == 9 and len(hi_joints) == 9, (lo_joints, hi_joints)
    pin_eff = [(lo_joints[w], hi_joints[w]) for w in range(9)]

    chunks = [(q, j, half_of_q[q]) for q, j in raw]

    # schedule: alternate lo/hi chunks, draining groups in order per class.
    # H-class group order chosen so chunks consume h1 windows roughly in the
    # order stage A evacuates them (window-ascending).
    by_q = {}
    for i, (q, j, half) in enumerate(chunks):
        by_q.setdefault(q, []).append(i)
    horder = [7, 3, 4, 6, 8, 0]
    lqueue = [i for q in sorted(by_q) if half_of_q[q] == 0 for i in by_q[q]]
    hqueue = [i for q in horder for i in by_q[q]]
    emitted = []
    li = hi_ = 0
    while li < len(lqueue) or hi_ < len(hqueue):
        if li < len(lqueue):
            emitted.append(lqueue[li])
            li += 1
        if hi_ < len(hqueue):
            emitted.append(hqueue[hi_])
            hi_ += 1
    return chunks, pin_eff, emitted


def _build_constants(A, W1, b1, W2, b2, W3, b3, Wp1, bp1, Wp2, bp2, chunks, pin_eff):
    A = np.asarray(A, np.float32)
    W1 = np.asarray(W1, np.float32)
    W2 = np.asarray(W2, np.float32)
    W3 = np.asarray(W3, np.float32)
    Wp1 = np.asarray(Wp1, np.float32)
    Wp2 = np.asarray(Wp2, np.float32)
    b1 = np.asarray(b1, np.float32)
    b2 = np.asarray(b2, np.float32)
    b3 = np.asarray(b3, np.float32)
    bp1 = np.asarray(bp1, np.float32)
    bp2 = np.asarray(bp2, np.float32)

    C1 = np.kron(A, W1)                 # (1088, 51)
    C2 = np.kron(A, W2)                 # (1088, 1088)
    C3 = np.kron(A, W3)
    D = Wp1 @ C3                        # (64, 1088)
    bp1p = Wp1 @ np.tile(b3, NJ) + bp1  # (64,)

    def rows_of(groups):
        return [np.concatenate([np.arange(j * H, (j + 1) * H) for j in g])
                for g in groups]

    rows_in = rows_of(pin_eff)
    rows_out = rows_of(POUT)
    bf = ml_dtypes.bfloat16

    # g1: stage A lhsT, window w = C1[rows_in[w]].T padded to 128 cols,
    # replicated at partitions 0-50 and 64-114.
    g1 = np.zeros((128, 9 * 128), np.float32)
    for w in range(9):
        blk = C1[rows_in[w]].T
        g1[0:51, 128 * w:128 * w + blk.shape[1]] = blk
        g1[64:115, 128 * w:128 * w + blk.shape[1]] = blk

    # g2: chunk i = (q, j, half): (64, Mq) block at rows 64*half.., col 128*i
    g2 = np.zeros((128, len(chunks) * 128), np.float32)
    for i, (q, j, half) in enumerate(chunks):
        blk = C2[np.ix_(rows_out[q], np.arange(j * H, (j + 1) * H))].T
        g2[64 * half:64 * half + 64, 128 * i:128 * i + blk.shape[1]] = blk

    # g3: chunk p: (Kp, 64) = D[:, rows_out[p]].T at col 128p, padded to 128
    # cols and 128 rows of zeros: keeps stage C in (128,128) tile mode (no
    # PE mode switch) and makes reading full h2 K-windows exact even where
    # h2 holds pad garbage.
    g3 = np.zeros((128, 9 * 128), np.float32)
    for p in range(9):
        blk = D[:, rows_out[p]].T
        g3[0:blk.shape[0], 128 * p:128 * p + blk.shape[1]] = blk

    wp2t = np.zeros((65, DOUT), np.float32)
    wp2t[0:64] = Wp2.T
    wp2t[64] = bp2

    return {
        "g1": g1.astype(bf),
        "g2": g2.astype(bf),
        "g3": g3.astype(bf),
        "wp2t": wp2t.astype(bf),
        "b1p": np.tile(b1, 2).reshape(128, 1).copy(),
        "b2p": np.tile(b2, 2).reshape(128, 1).copy(),
        "bp1p": bp1p.reshape(64, 1).copy(),
    }


def _build_program(chunks, slots, pin_eff, reps=1):
    win_of = {}
    for w, (lo, hi) in enumerate(pin_eff):
        win_of[(lo, 0)] = w
        win_of[(hi, 1)] = w
    nc = bacc.Bacc(None)

    x_d = nc.declare_dram_parameter("x2", [128, BC], BF16, isOutput=False)
    g1_d = nc.declare_dram_parameter("g1", [128, 9 * 128], BF16, isOutput=False)
    g2_d = nc.declare_dram_parameter("g2", [128, len(chunks) * 128], BF16,
                                     isOutput=False)
    g3_d = nc.declare_dram_parameter("g3", [128, 9 * 128], BF16, isOutput=False)
    wp2t_d = nc.declare_dram_parameter("wp2t", [65, DOUT], BF16, isOutput=False)
    b1p_d = nc.declare_dram_parameter("b1p", [128, 1], F32, isOutput=False)
    b2p_d = nc.declare_dram_parameter("b2p", [128, 1], F32, isOutput=False)
    bp1p_d = nc.declare_dram_parameter("bp1p", [H, 1], F32, isOutput=False)
    out_d = nc.declare_dram_parameter("out", [BC, DOUT], F32, isOutput=True)

    out_r = out_d.rearrange("(c p) f -> p c f", p=128)      # (128, 64, 256)

    AF = mybir.ActivationFunctionType
    ALU = mybir.AluOpType

    nb = int(os.environ.get("KERNEL_BCHUNKS", str(len(slots))))
    slots = slots[:nb]
    first_of_q, last_of_q = {}, {}
    for i in slots:
        q = chunks[i][0]
        first_of_q.setdefault(q, i)
        last_of_q[q] = i

    with tile.TileContext(nc) as tc:
        with (
            tc.tile_pool(name="const", bufs=1) as cp,
            tc.tile_pool(name="h1", bufs=2) as h1p,
            tc.tile_pool(name="h2", bufs=2) as h2p,
            tc.tile_pool(name="ot", bufs=3) as otp,
            tc.tile_pool(name="psa", bufs=3, space="PSUM") as psa,
            tc.tile_pool(name="psb", bufs=4, space="PSUM") as psb,
            tc.tile_pool(name="psc", bufs=1, space="PSUM") as psc,
        ):
            x_sb = cp.tile([128, BC], BF16)
            g1_sb = cp.tile([128, 9 * 128], BF16)
            g2_sb = cp.tile([128, len(chunks) * 128], BF16)
            g3_sb = cp.tile([128, 9 * 128], BF16)
            wp2t_sb = cp.tile([65, DOUT], BF16)
            b1p_sb = cp.tile([128, 1], F32)
            b2p_sb = cp.tile([128, 1], F32)
            bp1p_sb = cp.tile([H, 1], F32)
            z4_sb = cp.tile([65, 2 * TILE_N], BF16)   # manual double buffer

            nc.sync.dma_start(x_sb[:], x_d[:])
            nc.sync.dma_start(g2_sb[:], g2_d[:])
            nc.sync.dma_start(b2p_sb[:], b2p_d[:])
            nc.sync.dma_start(g1_sb[:], g1_d[:])
            nc.sync.dma_start(g3_sb[:], g3_d[:])
            nc.sync.dma_start(wp2t_sb[:], wp2t_d[:])
            nc.sync.dma_start(b1p_sb[:], b1p_d[:])
            nc.sync.dma_start(bp1p_sb[:], bp1p_d[:])
            nc.gpsimd.memset(z4_sb[64:65, :], 1.0)    # stage-D bias row

            evac_cycle = ["act", "dve", "act", "dve", "act"]
            evac_state = [0]

            def _next_eng():
                eng = evac_cycle[evac_state[0] % len(evac_cycle)]
                evac_state[0] += 1
                return eng

            def evac(dst, src, bias):
                eng = _next_eng()
                if eng == "act":
                    nc.scalar.activation(dst, src, AF.Relu, bias=bias)
                elif eng == "dve":
                    nc.vector.tensor_scalar(out=dst, in0=src, scalar1=bias,
                                            scalar2=0.0, op0=ALU.add,
                                            op1=ALU.max)
                else:
                    nc.gpsimd.tensor_scalar(out=dst, in0=src, scalar1=bias,
                                            scalar2=0.0, op0=ALU.add,
                                            op1=ALU.max)

            def evac_copy(dst, src):
                eng = _next_eng()
                if eng == "act":
                    nc.scalar.copy(dst, src)
                elif eng == "dve":
                    nc.vector.tensor_copy(dst, src)
                else:
                    nc.gpsimd.tensor_copy(dst, src)

            nopack_a = bool(int(os.environ.get("KERNEL_NOPACK_A", "0")))
            nopack_b = bool(int(os.environ.get("KERNEL_NOPACK_B", "0")))
            h2_tiles = {}

            def stage_ab(t):
                xt_lo = x_sb[0:51, TILE_N * t:TILE_N * (t + 1)]
                xt_hi = x_sb[64:115, TILE_N * t:TILE_N * (t + 1)]

                # ---- stage A
                h1_sb = h1p.tile([128, 9 * TILE_N], BF16)
                for s in range(5):
                    w0 = 2 * s
                    group = []
                    for k in range(1 if w0 == 8 else 2):
                        w = w0 + k
                        ps = psa.tile([128, TILE_N], F32)
                        group.append((w, ps))
                        if nopack_a:
                            nc.tensor.matmul(
                                ps[:, :], g1_sb[0:51, 128 * w:128 * (w + 1)],
                                xt_lo, start=True, stop=True,
                            )
                        elif k == 0:
                            nc.tensor.matmul(
                                ps[:, :], g1_sb[0:51, 128 * w:128 * (w + 1)],
                                xt_lo, start=True, stop=True,
                                tile_position=(0, 0),
                            )
                        else:
                            nc.tensor.matmul(
                                ps[:, :], g1_sb[64:115, 128 * w:128 * (w + 1)],
                                xt_hi, start=True, stop=True,
                                tile_position=(64, 0),
                            )
                    for w, ps in group:
                        evac(h1_sb[:, TILE_N * w:TILE_N * (w + 1)],
                             ps[:, :], b1p_sb[:])

                # ---- stage B
                h2_sb = h2p.tile([128, 9 * TILE_N], BF16)
                h2_tiles[t] = h2_sb
                qtile = {}
                for i in slots:
                    q, j, half = chunks[i]
                    if q not in qtile:
                        qtile[q] = psb.tile([128, TILE_N], F32, name="ps_q")
                    ps = qtile[q]
                    w = win_of[(j, half)]
                    r0 = 64 * half
                    if nopack_b:
                        # K=128 read; the unused half of the g2 chunk is zero
                        nc.tensor.matmul(
                            ps[:, :],
                            g2_sb[0:128, 128 * i:128 * (i + 1)],
                            h1_sb[0:128, TILE_N * w:TILE_N * (w + 1)],
                            start=(i == first_of_q[q]),
                            stop=(i == last_of_q[q]),
                        )
                    else:
                        nc.tensor.matmul(
                            ps[:, :],
                            g2_sb[r0:r0 + 64, 128 * i:128 * (i + 1)],
                            h1_sb[r0:r0 + 64, TILE_N * w:TILE_N * (w + 1)],
                            start=(i == first_of_q[q]),
                            stop=(i == last_of_q[q]),
                            tile_position=(r0, 0),
                        )
                    if i == last_of_q[q]:
                        evac(h2_sb[:, TILE_N * q:TILE_N * (q + 1)],
                             ps[:, :], b2p_sb[:])

            def stage_c(t):
                h2_sb = h2_tiles.pop(t)
                ps_c = psc.tile([128, TILE_N], F32)
                for p in range(9):
                    nc.tensor.matmul(
                        ps_c[:, :],
                        g3_sb[:, 128 * p:128 * (p + 1)],
                        h2_sb[:, TILE_N * p:TILE_N * (p + 1)],
                        start=(p == 0), stop=(p == 8),
                    )
                zoff = TILE_N * (t % 2)
                evac(z4_sb[0:64, zoff:zoff + TILE_N], ps_c[0:64, :], bp1p_sb[:])

            def stage_d(t):
                zoff = TILE_N * (t % 2)
                ot_sb = otp.tile([128, 4 * DOUT], F32)
                for half in range(2):
                    ps_d = psb.tile([128, 2 * DOUT], F32, name="ps_q")
                    for k in range(2):
                        kk = 2 * half + k
                        nc.tensor.matmul(
                            ps_d[:, DOUT * k:DOUT * (k + 1)],
                            z4_sb[0:65, zoff + 128 * kk:zoff + 128 * (kk + 1)],
                            wp2t_sb[:],
                            start=True, stop=True,
                        )
                    evac_copy(ot_sb[:, 2 * DOUT * half:2 * DOUT * (half + 1)],
                              ps_d[:])
                nc.sync.dma_start(
                    out_r[:, 4 * t:4 * (t + 1), :],
                    ot_sb[:],
                )

            ntiles = int(os.environ.get("KERNEL_NTILES", str(NTILES)))

            def body():
                # software pipeline: A/B lead, C one tile behind, D two —
                # the PE never waits on a just-issued evacuation.
                for g in range(ntiles + 2):
                    if g < ntiles:
                        stage_ab(g)
                    if 1 <= g <= ntiles:
                        stage_c(g - 1)
                    if 2 <= g:
                        stage_d(g - 2)

            if reps == 1:
                body()
            else:
                with tc.For_i(0, reps, 1):
                    body()

    nc.compile()
    return nc


_CACHE = {}


def kernel(**inputs):
    global LAST_RESULTS
    x = np.ascontiguousarray(np.asarray(inputs["x"], np.float32))
    chunks, pin_eff, slots = _plan()
    consts = _build_constants(
        inputs["A"], inputs["W1"], inputs["b1"], inputs["W2"], inputs["b2"],
        inputs["W3"], inputs["b3"], inputs["Wp1"], inputs["bp1"],
        inputs["Wp2"], inputs["bp2"], chunks, pin_eff,
    )

    reps = int(os.environ.get("BENCH_REPS", "1"))
    key = (reps,)
    if key not in _CACHE:
        _CACHE[key] = _build_program(chunks, slots, pin_eff, reps=reps)
    nc = _CACHE[key]

    xf = x.reshape(B_TOTAL, NJ * DIN)
    in_maps = []
    for c in range(NCORES):
        xT = xf[c * BC:(c + 1) * BC].T.astype(ml_dtypes.bfloat16)
        x2 = np.zeros((128, BC), ml_dtypes.bfloat16)
        x2[0:51] = xT
        x2[64:115] = xT
        m = dict(consts)
        m["x2"] = x2
        in_maps.append(m)

    res = run_bass_kernel_spmd(nc, in_maps, list(range(NCORES)))
    LAST_RESULTS = res
    out = np.concatenate([res.results[c]["out"] for c in range(NCORES)], axis=0)
    return out.astype(np.float32, copy=False)


if __name__ == "__main__":
    chunks, pin_eff, slots = _plan()
    nlo = sum(1 for c in chunks if c[2] == 0)
    print(f"chunks={len(chunks)} lo={nlo} hi={len(chunks) - nlo}")
    print("emission:", [(chunks[i][0], chunks[i][1], "LH"[chunks[i][2]]) for i in slots])

